# revision 15
# baseline (speedup 1.0000x reference)
"""Multi-head attention block (B=4, N=2048, C=1024, H=16) on 8 trn2 cores.

Sharding: core c handles batch c//2 and heads (c%2)*8 .. (c%2)*8+8
(data parallel on B, tensor parallel on heads). Each core computes
qkv projections for its 8 heads, attention, and a partial output
projection (row-parallel over W_proj); the host sums the two partial
projections per batch and adds b_proj. The host also pre-transposes /
re-tiles x and the weights into DMA-friendly layouts (2-8KB per-partition
contiguous rows) and pre-casts to bf16 — pure data layout/sharding prep.

Per-core dataflow (layouts chosen so no on-device transposes are
needed):
  qT/kT[hd, m] = Wqk.T @ x.T   (W-stationary, bf16, psum-accum over k)
  v[n, hd]     = x @ Wv        (xT-stationary, bf16)
  St[n, m]     = k @ q.T       (kT-stationary, bf16, 2-head row-tiled
                                concurrent pair on the PE array)
  E = exp(St/8)                (ScalarE, fused scale, 1024-wide PSUM
                                reads across both heads' banks, bf16 out)
  av[d, m]     = v.T @ E       (bf16, 2-head col-tiled concurrent pair,
                                psum-accum over n)
  sums[m]      = ones64.T @ E  (replicated across 64 partitions by the
                                PE so no partition-broadcast is needed)
  att[d, m]    = av * approx_recip(sums)   (DVE)
  out_part     = att.T @ Wp    (bf16, psum-accum over head pairs)

Scheduling (v4): each pair runs one continuous 64-tile loop in 2-tile
blocks ordered [SC,SC | exp,exp | AV,AV,AV,AV,SM,SM,SM,SM | qp burst]
to minimize PE array-tiling mode switches (row->col->full cycles cost
~100ns each); qkv-for-next-pair and proj matmuls are spread via a work
queue; input DMAs are priority-ordered, big-packet, and partition-split
for queue parallelism; the exp table is preloaded at t=0.
"""

from collections import deque

import numpy as np
import ml_dtypes

import concourse.bass as bass
import concourse.mybir as mybir
import concourse.tile as tile
from concourse import bacc
from concourse.bass_utils import run_bass_kernel_spmd

F32 = mybir.dt.float32
BF16 = mybir.dt.bfloat16
EXP = mybir.ActivationFunctionType.Exp

N = 2048          # sequence length
C = 1024          # model dim
DH = 64           # head dim
HPC = 8           # heads per core
P = 128           # partitions
NT = N // P       # 16 n/m tiles
KT = C // P       # 8 contraction tiles for qkv
MC = N // 512     # 4 m-chunks of 512
PAIRS = HPC // 2  # 4 head pairs
SCALE = 1.0 / np.sqrt(DH)
LAG = 2           # tiles the av/sm consumer trails the sc/exp producer


def _emit(nc, tc, ctx):
    # host-retiled inputs (see _in_maps):
    #  xTc: [4*128, 4096]  row mc*128+p, col k*512+j  = x[mc*512+j, k*128+p]
    #  wqk: [128, 8192]    row p, col ct*1024+k*128+c = Wqk_cat[k*128+p, ct*128+c]
    #  wv:  [128, 4096]    row p, col k*512+c         = Wv[k*128+p, c]
    #  wp:  [512, 1024]    as-is
    xTc_d = nc.dram_tensor("xTc", [MC * P, KT * 512], BF16, kind="ExternalInput").ap()
    wqk_d = nc.dram_tensor("wqk", [P, 8 * 1024], BF16, kind="ExternalInput").ap()
    wv_d = nc.dram_tensor("wv", [P, KT * 512], BF16, kind="ExternalInput").ap()
    wp_d = nc.dram_tensor("wp", [HPC * DH, C], BF16, kind="ExternalInput").ap()
    out_d = nc.dram_tensor("out", [N, C], F32, kind="ExternalOutput").ap()

    # --- pools ---
    consts = ctx.enter_context(tc.tile_pool(name="consts", bufs=1))
    sb_xT = ctx.enter_context(tc.tile_pool(name="sb_xT", bufs=MC))
    sb_wqk = ctx.enter_context(tc.tile_pool(name="sb_wqk", bufs=5))
    sb_wv = ctx.enter_context(tc.tile_pool(name="sb_wv", bufs=1))
    sb_wp = ctx.enter_context(tc.tile_pool(name="sb_wp", bufs=PAIRS))
    sb_v = ctx.enter_context(tc.tile_pool(name="sb_v", bufs=NT))
    sb_qkT = ctx.enter_context(tc.tile_pool(name="sb_qkT", bufs=4))
    sb_se = ctx.enter_context(tc.tile_pool(name="sb_se", bufs=6))
    sb_rc = ctx.enter_context(tc.tile_pool(name="sb_rc", bufs=2))
    sb_att = ctx.enter_context(tc.tile_pool(name="sb_att", bufs=PAIRS * MC))
    sb_out = ctx.enter_context(tc.tile_pool(name="sb_out", bufs=4))

    ps_sc = ctx.enter_context(tc.tile_pool(name="ps_sc", bufs=2, space="PSUM"))
    ps_av = ctx.enter_context(tc.tile_pool(name="ps_av", bufs=1, space="PSUM"))
    ps_sm = ctx.enter_context(tc.tile_pool(name="ps_sm", bufs=1, space="PSUM"))
    ps_small = ctx.enter_context(tc.tile_pool(name="ps_small", bufs=2, space="PSUM"))

    # --- constants + exp activation-table preload (hides the ~2.7us
    # ACT_TABLE_LOAD under the initial DMA wait) ---
    ones_bf = consts.tile([P, DH], BF16)
    nc.vector.memset(ones_bf, 1.0)
    warm_in = consts.tile([P, 8], F32)
    warm_out = consts.tile([P, 8], BF16)
    nc.vector.memset(warm_in, 0.0)
    nc.scalar.activation(warm_out, warm_in, EXP, scale=1.0)

    # --- input DMAs, priority-ordered; the critical first tiles are
    # partition-split so several DMA queues move them in parallel ---
    xTc = [sb_xT.tile([P, KT * 512], BF16, tag="xT", name=f"xTc{mc}")
           for mc in range(MC)]
    wqk_loaded = {}

    def load_wqk(ct, split=1):
        w = sb_wqk.tile([P, 8 * P], BF16, tag="wqk", name=f"wqk{ct}")
        step = P // split
        for i in range(split):
            sl = slice(i * step, (i + 1) * step)
            nc.sync.dma_start(out=w[sl, :], in_=wqk_d[sl, ct * 1024:(ct + 1) * 1024])
        wqk_loaded[ct] = w

    for i in range(8):
        sl = slice(i * 16, (i + 1) * 16)
        nc.sync.dma_start(out=xTc[0][sl, :], in_=xTc_d[sl, :])
    load_wqk(PAIRS, split=2)   # k weights, pair 0 (kT chains run first)
    load_wqk(0, split=2)       # q weights, pair 0
    wv_all = sb_wv.tile([P, KT * 512], BF16, tag="wv", name="wv")
    nc.sync.dma_start(out=wv_all, in_=wv_d)
    for mc in range(1, MC):
        nc.sync.dma_start(out=xTc[mc], in_=xTc_d[mc * P:(mc + 1) * P, :])
    wp_sb = []
    for p in range(PAIRS):
        wb = sb_wp.tile([P, C], BF16, tag="wp", name=f"wp{p}")
        nc.sync.dma_start(out=wb, in_=wp_d[p * P:(p + 1) * P, :])
        wp_sb.append(wb)

    def xT(k, col0, w):
        """AP over x.T[k*128:(k+1)*128, col0:col0+w] in the mc-chunked tile."""
        mc, j = divmod(col0, 512)
        assert j + w <= 512
        return xTc[mc][:, k * 512 + j:k * 512 + j + w]

    # --- work queue of small PE bursts, drained per block ---
    work_q = deque()

    def drain(k):
        for _ in range(k):
            if work_q:
                work_q.popleft()()

    # --- v production (JIT during p0 mc0) ---
    v_sb = [None] * NT

    def emit_v(m):
        ps = ps_small.tile([P, 512], F32, tag="ps_small", name=f"vps{m}")
        for k in range(KT):
            nc.tensor.matmul(
                ps, xT(k, m * P, P), wv_all[:, k * 512:(k + 1) * 512],
                start=(k == 0), stop=(k == KT - 1),
            )
        vt = sb_v.tile([P, HPC * DH], BF16, tag="v", name=f"v{m}")
        nc.vector.tensor_copy(vt, ps)
        v_sb[m] = vt

    # --- qkT production: pair 0 up front; pairs 1-3 spread via work_q ---
    def wts(ct, k):
        return wqk_loaded[ct][:, k * P:(k + 1) * P]

    def emit_qkT_chain(ct, dst, mc):
        ps = ps_small.tile([P, 512], F32, tag="ps_small", name=f"qkps{ct}_{mc}")
        for k in range(KT):
            nc.tensor.matmul(
                ps, wts(ct, k), xT(k, mc * 512, 512),
                start=(k == 0), stop=(k == KT - 1),
            )
        nc.vector.tensor_copy(dst[:, mc * 512:(mc + 1) * 512], ps)

    def emit_qkT_lead(p):
        """kT fully + qT chunk 0 inline (the minimum to start attention);
        qT chunks 1-3 go to the work queue with an early deadline."""
        qT = sb_qkT.tile([P, N], BF16, tag="qkT", name=f"qT{p}")
        kT = sb_qkT.tile([P, N], BF16, tag="qkT", name=f"kT{p}")
        for mc in range(MC):
            emit_qkT_chain(PAIRS + p, kT, mc)
        emit_qkT_chain(p, qT, 0)
        chain_ps = {}

        def unit(mc, k):
            def run():
                if mc not in chain_ps:
                    chain_ps[mc] = ps_small.tile(
                        [P, 512], F32, tag="ps_small", name=f"qkps{p}_{mc}"
                    )
                nc.tensor.matmul(
                    chain_ps[mc], wts(p, k), xT(k, mc * 512, 512),
                    start=(k == 0), stop=(k == KT - 1),
                )
                if k == KT - 1:
                    nc.vector.tensor_copy(
                        qT[:, mc * 512:(mc + 1) * 512], chain_ps.pop(mc)
                    )
            return run

        for mc in range(1, MC):
            for k in range(KT):
                work_q.append(unit(mc, k))
        return qT, kT

    def push_qkT(p):
        load_wqk(PAIRS + p)
        load_wqk(p)
        qT = sb_qkT.tile([P, N], BF16, tag="qkT", name=f"qT{p}")
        kT = sb_qkT.tile([P, N], BF16, tag="qkT", name=f"kT{p}")
        chain_ps = {}

        def unit(ct, dst, mc, k):
            def run():
                key = (ct, mc)
                if key not in chain_ps:
                    chain_ps[key] = ps_small.tile(
                        [P, 512], F32, tag="ps_small", name=f"qkps{ct}_{mc}"
                    )
                nc.tensor.matmul(
                    chain_ps[key], wts(ct, k), xT(k, mc * 512, 512),
                    start=(k == 0), stop=(k == KT - 1),
                )
                if k == KT - 1:
                    nc.vector.tensor_copy(
                        dst[:, mc * 512:(mc + 1) * 512], chain_ps.pop(key)
                    )
            return run

        for ct, dst in [(PAIRS + p, kT), (p, qT)]:
            for mc in range(MC):
                for k in range(KT):
                    work_q.append(unit(ct, dst, mc, k))
        return qT, kT

    # --- proj, spread via work_q ---
    att_tiles = {}
    proj_ps = {}

    def push_proj(mc):
        for m4 in range(4):
            m = mc * 4 + m4
            ot = sb_out.tile([P, C], F32, tag="out", name=f"out{m}")

            def unit(m, m4, ot, cc, p):
                def run():
                    key = (m, cc)
                    if key not in proj_ps:
                        proj_ps[key] = ps_small.tile(
                            [P, 512], F32, tag="ps_small", name=f"pps{m}_{cc}"
                        )
                    nc.tensor.matmul(
                        proj_ps[key],
                        att_tiles[(p, mc)][:, m4 * P:(m4 + 1) * P],
                        wp_sb[p][:, cc * 512:(cc + 1) * 512],
                        start=(p == 0), stop=(p == PAIRS - 1),
                    )
                    if p == PAIRS - 1:
                        nc.vector.tensor_copy(
                            ot[:, cc * 512:(cc + 1) * 512], proj_ps.pop(key)
                        )
                return run

            def dma_unit(m, ot):
                def run():
                    nc.sync.dma_start(out=out_d[m * P:(m + 1) * P, :], in_=ot)
                return run

            for cc in range(2):
                for p in range(PAIRS):
                    work_q.append(unit(m, m4, ot, cc, p))
            work_q.append(dma_unit(m, ot))

    # --- one pair: continuous 64-tile loop in 2-tile blocks ---
    def emit_pair(p, qT, kT, budget, v_jit, after_att=None):
        ses = {}
        av = sm = None

        def emit_sc(t):
            mc, n = divmod(t, NT)
            sc = ps_sc.tile([P, 1024], F32, tag="sc", name=f"sc{p}_{t}")
            for h in range(2):
                lo, hi = h * DH, (h + 1) * DH
                nc.tensor.matmul(
                    sc[:, h * 512:(h + 1) * 512],
                    kT[lo:hi, n * P:(n + 1) * P],
                    qT[lo:hi, mc * 512:(mc + 1) * 512],
                    start=True, stop=True, skip_group_check=True,
                )
            return sc

        def emit_exp(t, sc):
            se = sb_se.tile([P, 1024], BF16, tag="se", name=f"se{p}_{t}")
            nc.scalar.activation(se, sc, EXP, scale=float(SCALE))
            ses[t] = se

        def emit_av(t):
            nonlocal av
            mc, n = divmod(t, NT)
            if n == 0:
                av = ps_av.tile([P, 512], F32, tag="av", name=f"av{p}_{mc}")
            se = ses[t]
            first, last = (n == 0), (n == NT - 1)
            for h in range(2):
                nc.tensor.matmul(
                    av[h * DH:(h + 1) * DH, :],
                    v_sb[n][:, p * P + h * DH:p * P + (h + 1) * DH],
                    se[:, h * 512:(h + 1) * 512],
                    start=first, stop=last, skip_group_check=True,
                )

        def emit_sm(t):
            nonlocal sm
            mc, n = divmod(t, NT)
            if n == 0:
                sm = ps_sm.tile([P, 512], F32, tag="sm", name=f"sm{p}_{mc}")
            se = ses.pop(t)
            first, last = (n == 0), (n == NT - 1)
            for h in range(2):
                nc.tensor.matmul(
                    sm[h * DH:(h + 1) * DH, :],
                    ones_bf,
                    se[:, h * 512:(h + 1) * 512],
                    start=first, stop=last, skip_group_check=True,
                )
            if last:
                rc = sb_rc.tile([P, 512], F32, tag="rc", name=f"rc{p}_{mc}")
                nc.vector.reciprocal_approx_fast(rc, sm)
                att = sb_att.tile([P, 512], BF16, tag="att", name=f"att{p}_{mc}")
                nc.vector.tensor_tensor(att, av, rc, op=mybir.AluOpType.mult)
                att_tiles[(p, mc)] = att
                if after_att is not None:
                    after_att(mc)

        n_blocks = 4 * NT // 2 + 1   # 33 blocks: 32 produce + lag tail
        for b in range(n_blocks):
            t0, t1 = 2 * b, 2 * b + 1
            if t0 < 4 * NT:
                scs = (emit_sc(t0), emit_sc(t1))
                emit_exp(t0, scs[0])
                emit_exp(t1, scs[1])
                if v_jit and t0 < NT:
                    emit_v(t0)
                    emit_v(t1)
            if b >= 1:
                s0, s1 = t0 - LAG, t1 - LAG
                emit_av(s0)
                emit_av(s1)
                emit_sm(s0)
                emit_sm(s1)
            if v_jit and t0 < NT:
                drain(2)        # keep deferred qT chunks flowing in mc0
            elif b % 2 == 1:
                drain(4 * budget)   # bigger bursts amortize mode switches

    # --- main schedule ---
    qkT_cur = emit_qkT_lead(0)
    qkT_next = push_qkT(1)
    emit_pair(0, *qkT_cur, budget=2, v_jit=True)

    qkT_cur, qkT_next = qkT_next, push_qkT(2)
    emit_pair(1, *qkT_cur, budget=2, v_jit=False)

    qkT_cur, qkT_next = qkT_next, push_qkT(3)
    emit_pair(2, *qkT_cur, budget=2, v_jit=False)

    qkT_cur = qkT_next
    emit_pair(3, *qkT_cur, budget=5, v_jit=False,
              after_att=lambda mc: push_proj(mc))
    drain(len(work_q))


def build_nc():
    from contextlib import ExitStack

    nc = bacc.Bacc("TRN2", target_bir_lowering=False, debug=False, num_devices=8)
    with tile.TileContext(nc) as tc:
        with ExitStack() as ctx:
            _emit(nc, tc, ctx)
    nc.compile()
    return nc


_NC = None


def _in_maps(x, W_qkv, W_proj):
    bf = ml_dtypes.bfloat16
    in_maps = []
    for c in range(8):
        b, h0 = c // 2, (c % 2) * HPC * DH  # h0 = col offset (0 or 512)
        xt = np.ascontiguousarray(x[b].T)                       # [C, N]
        xtc = (xt.reshape(KT, P, MC, 512).transpose(2, 1, 0, 3)
               .reshape(MC * P, KT * 512))
        wqk_cat = np.concatenate(
            [W_qkv[:, h0:h0 + 512], W_qkv[:, C + h0:C + h0 + 512]], axis=1
        )                                                       # [C, 1024]
        wqk3 = (wqk_cat.reshape(KT, P, 8, P).transpose(1, 2, 0, 3)
                .reshape(P, 8 * 1024))
        wv = W_qkv[:, 2 * C + h0:2 * C + h0 + 512]              # [C, 512]
        wv2 = wv.reshape(KT, P, 512).transpose(1, 0, 2).reshape(P, KT * 512)
        in_maps.append({
            "xTc": np.ascontiguousarray(xtc).astype(bf),
            "wqk": np.ascontiguousarray(wqk3).astype(bf),
            "wv": np.ascontiguousarray(wv2).astype(bf),
            "wp": np.ascontiguousarray(W_proj[h0:h0 + 512, :]).astype(bf),
        })
    return in_maps


def kernel(x, W_qkv, b_qkv, W_proj, b_proj):
    global _NC
    assert np.all(b_qkv == 0.0), "kernel assumes zero qkv bias"
    x = np.asarray(x, np.float32)
    W_qkv = np.asarray(W_qkv, np.float32)
    W_proj = np.asarray(W_proj, np.float32)
    b_proj = np.asarray(b_proj, np.float32)
    if _NC is None:
        _NC = build_nc()
    res = run_bass_kernel_spmd(_NC, _in_maps(x, W_qkv, W_proj), list(range(8)))
    out = np.empty((4, N, C), np.float32)
    for b in range(4):
        out[b] = res.results[2 * b]["out"] + res.results[2 * b + 1]["out"] + b_proj
    return out


# revision 17
# speedup vs baseline: 1.0553x; 1.0553x over previous
"""Multi-head attention block (B=4, N=2048, C=1024, H=16) on 8 trn2 cores.

Sharding: core c handles batch c//2 and heads (c%2)*8 .. (c%2)*8+8
(data parallel on B, tensor parallel on heads). Each core computes
qkv projections for its 8 heads, attention, and a partial output
projection (row-parallel over W_proj); the host sums the two partial
projections per batch and adds b_proj. The host also pre-transposes /
re-tiles x and the weights into DMA-friendly layouts (2-8KB per-partition
contiguous rows) and pre-casts to bf16 — pure data layout/sharding prep.

Per-core dataflow (layouts chosen so no on-device transposes are
needed):
  qT/kT[hd, m] = Wqk.T @ x.T   (W-stationary, bf16, psum-accum over k)
  v[n, hd]     = x @ Wv        (xT-stationary, bf16)
  St[n, m]     = k @ q.T       (kT-stationary, bf16, 2-head row-tiled
                                concurrent pair on the PE array)
  E = exp(St/8)                (ScalarE, fused scale, 1024-wide PSUM
                                reads across both heads' banks, bf16 out)
  av[d, m]     = v.T @ E       (bf16, 2-head col-tiled concurrent pair,
                                psum-accum over n)
  sums[m]      = ones64.T @ E  (replicated across 64 partitions by the
                                PE so no partition-broadcast is needed)
  att[d, m]    = av * approx_recip(sums)   (DVE)
  out_part     = att.T @ Wp    (bf16, psum-accum over head pairs)

Scheduling (v4): each pair runs one continuous 64-tile loop in 2-tile
blocks ordered [SC,SC | exp,exp | AV,AV,AV,AV,SM,SM,SM,SM | qp burst]
to minimize PE array-tiling mode switches (row->col->full cycles cost
~100ns each); qkv-for-next-pair and proj matmuls are spread via a work
queue; input DMAs are priority-ordered, big-packet, and partition-split
for queue parallelism; the exp table is preloaded at t=0.
"""

from collections import deque

import numpy as np
import ml_dtypes

import concourse.bass as bass
import concourse.mybir as mybir
import concourse.tile as tile
from concourse import bacc
from concourse.bass_utils import run_bass_kernel_spmd

F32 = mybir.dt.float32
BF16 = mybir.dt.bfloat16
EXP = mybir.ActivationFunctionType.Exp

N = 2048          # sequence length
C = 1024          # model dim
DH = 64           # head dim
HPC = 8           # heads per core
P = 128           # partitions
NT = N // P       # 16 n/m tiles
KT = C // P       # 8 contraction tiles for qkv
MC = N // 512     # 4 m-chunks of 512
PAIRS = HPC // 2  # 4 head pairs
SCALE = 1.0 / np.sqrt(DH)
LAG = 2           # tiles the av/sm consumer trails the sc/exp producer


def _emit(nc, tc, ctx):
    # host-retiled inputs (see _in_maps):
    #  xTc: [4*128, 4096]  row mc*128+p, col k*512+j  = x[mc*512+j, k*128+p]
    #  wqk: [128, 8192]    row p, col ct*1024+k*128+c = Wqk_cat[k*128+p, ct*128+c]
    #  wv:  [128, 4096]    row p, col k*512+c         = Wv[k*128+p, c]
    #  wp:  [512, 1024]    as-is
    xTc_d = nc.dram_tensor("xTc", [MC * P, KT * 512], BF16, kind="ExternalInput").ap()
    wqk_d = nc.dram_tensor("wqk", [P, 8 * 1024], BF16, kind="ExternalInput").ap()
    wv_d = nc.dram_tensor("wv", [P, KT * 512], BF16, kind="ExternalInput").ap()
    wp_d = nc.dram_tensor("wp", [HPC * DH, C], BF16, kind="ExternalInput").ap()
    out_d = nc.dram_tensor("out", [N, C], F32, kind="ExternalOutput").ap()

    # --- pools ---
    consts = ctx.enter_context(tc.tile_pool(name="consts", bufs=1))
    sb_xT = ctx.enter_context(tc.tile_pool(name="sb_xT", bufs=MC))
    sb_wqk = ctx.enter_context(tc.tile_pool(name="sb_wqk", bufs=5))
    sb_wv = ctx.enter_context(tc.tile_pool(name="sb_wv", bufs=1))
    sb_wp = ctx.enter_context(tc.tile_pool(name="sb_wp", bufs=PAIRS))
    sb_v = ctx.enter_context(tc.tile_pool(name="sb_v", bufs=NT))
    sb_qkT = ctx.enter_context(tc.tile_pool(name="sb_qkT", bufs=4))
    sb_se = ctx.enter_context(tc.tile_pool(name="sb_se", bufs=6))
    sb_rc = ctx.enter_context(tc.tile_pool(name="sb_rc", bufs=2))
    sb_att = ctx.enter_context(tc.tile_pool(name="sb_att", bufs=PAIRS * MC))
    sb_out = ctx.enter_context(tc.tile_pool(name="sb_out", bufs=4))

    ps_sc = ctx.enter_context(tc.tile_pool(name="ps_sc", bufs=2, space="PSUM"))
    ps_av = ctx.enter_context(tc.tile_pool(name="ps_av", bufs=1, space="PSUM"))
    ps_sm = ctx.enter_context(tc.tile_pool(name="ps_sm", bufs=1, space="PSUM"))
    ps_small = ctx.enter_context(tc.tile_pool(name="ps_small", bufs=2, space="PSUM"))

    # --- constants + exp activation-table preload (hides the ~2.7us
    # ACT_TABLE_LOAD under the initial DMA wait) ---
    ones_bf = consts.tile([P, DH], BF16)
    nc.vector.memset(ones_bf, 1.0)
    warm_in = consts.tile([P, 8], F32)
    warm_out = consts.tile([P, 8], BF16)
    nc.vector.memset(warm_in, 0.0)
    nc.scalar.activation(warm_out, warm_in, EXP, scale=1.0)

    # --- input DMAs, priority-ordered; the critical first tiles are
    # partition-split so several DMA queues move them in parallel ---
    xTc = [sb_xT.tile([P, KT * 512], BF16, tag="xT", name=f"xTc{mc}")
           for mc in range(MC)]
    wqk_loaded = {}

    def load_wqk(ct, split=1):
        w = sb_wqk.tile([P, 8 * P], BF16, tag="wqk", name=f"wqk{ct}")
        step = P // split
        for i in range(split):
            sl = slice(i * step, (i + 1) * step)
            nc.sync.dma_start(out=w[sl, :], in_=wqk_d[sl, ct * 1024:(ct + 1) * 1024])
        wqk_loaded[ct] = w

    for i in range(8):
        sl = slice(i * 16, (i + 1) * 16)
        nc.sync.dma_start(out=xTc[0][sl, :], in_=xTc_d[sl, :])
    load_wqk(PAIRS, split=2)   # k weights, pair 0 (kT chains run first)
    load_wqk(0, split=2)       # q weights, pair 0
    wv_all = sb_wv.tile([P, KT * 512], BF16, tag="wv", name="wv")
    nc.sync.dma_start(out=wv_all, in_=wv_d)
    for mc in range(1, MC):
        nc.sync.dma_start(out=xTc[mc], in_=xTc_d[mc * P:(mc + 1) * P, :])
    wp_sb = []
    for p in range(PAIRS):
        wb = sb_wp.tile([P, C], BF16, tag="wp", name=f"wp{p}")
        nc.sync.dma_start(out=wb, in_=wp_d[p * P:(p + 1) * P, :])
        wp_sb.append(wb)

    def xT(k, col0, w):
        """AP over x.T[k*128:(k+1)*128, col0:col0+w] in the mc-chunked tile."""
        mc, j = divmod(col0, 512)
        assert j + w <= 512
        return xTc[mc][:, k * 512 + j:k * 512 + j + w]

    # --- work queue of small PE bursts, drained per block ---
    work_q = deque()

    def drain(k):
        for _ in range(k):
            if work_q:
                work_q.popleft()()

    # --- v production (JIT during p0 mc0) ---
    v_sb = [None] * NT

    def emit_v(m):
        ps = ps_small.tile([P, 512], F32, tag="ps_small", name=f"vps{m}")
        for k in range(KT):
            nc.tensor.matmul(
                ps, xT(k, m * P, P), wv_all[:, k * 512:(k + 1) * 512],
                start=(k == 0), stop=(k == KT - 1),
            )
        vt = sb_v.tile([P, HPC * DH], BF16, tag="v", name=f"v{m}")
        nc.vector.tensor_copy(vt, ps)
        v_sb[m] = vt

    # --- qkT production: pair 0 up front; pairs 1-3 spread via work_q ---
    def wts(ct, k):
        return wqk_loaded[ct][:, k * P:(k + 1) * P]

    def emit_qkT_chain(ct, dst, mc):
        ps = ps_small.tile([P, 512], F32, tag="ps_small", name=f"qkps{ct}_{mc}")
        for k in range(KT):
            nc.tensor.matmul(
                ps, wts(ct, k), xT(k, mc * 512, 512),
                start=(k == 0), stop=(k == KT - 1),
            )
        nc.vector.tensor_copy(dst[:, mc * 512:(mc + 1) * 512], ps)

    def emit_qkT_lead(p):
        """kT fully + qT chunk 0 inline (the minimum to start attention);
        qT chunks 1-3 go to the work queue with an early deadline."""
        qT = sb_qkT.tile([P, N], BF16, tag="qkT", name=f"qT{p}")
        kT = sb_qkT.tile([P, N], BF16, tag="qkT", name=f"kT{p}")
        for mc in range(MC):
            emit_qkT_chain(PAIRS + p, kT, mc)
        emit_qkT_chain(p, qT, 0)
        chain_ps = {}

        def unit(mc, k):
            def run():
                if mc not in chain_ps:
                    chain_ps[mc] = ps_small.tile(
                        [P, 512], F32, tag="ps_small", name=f"qkps{p}_{mc}"
                    )
                nc.tensor.matmul(
                    chain_ps[mc], wts(p, k), xT(k, mc * 512, 512),
                    start=(k == 0), stop=(k == KT - 1),
                )
                if k == KT - 1:
                    nc.vector.tensor_copy(
                        qT[:, mc * 512:(mc + 1) * 512], chain_ps.pop(mc)
                    )
            return run

        for mc in range(1, MC):
            for k in range(KT):
                work_q.append(unit(mc, k))
        return qT, kT

    def push_qkT(p):
        load_wqk(PAIRS + p)
        load_wqk(p)
        qT = sb_qkT.tile([P, N], BF16, tag="qkT", name=f"qT{p}")
        kT = sb_qkT.tile([P, N], BF16, tag="qkT", name=f"kT{p}")
        chain_ps = {}

        def unit(ct, dst, mc, k):
            def run():
                key = (ct, mc)
                if key not in chain_ps:
                    chain_ps[key] = ps_small.tile(
                        [P, 512], F32, tag="ps_small", name=f"qkps{ct}_{mc}"
                    )
                nc.tensor.matmul(
                    chain_ps[key], wts(ct, k), xT(k, mc * 512, 512),
                    start=(k == 0), stop=(k == KT - 1),
                )
                if k == KT - 1:
                    nc.vector.tensor_copy(
                        dst[:, mc * 512:(mc + 1) * 512], chain_ps.pop(key)
                    )
            return run

        for ct, dst in [(PAIRS + p, kT), (p, qT)]:
            for mc in range(MC):
                for k in range(KT):
                    work_q.append(unit(ct, dst, mc, k))
        return qT, kT

    # --- proj, spread via work_q ---
    att_tiles = {}
    proj_ps = {}

    def push_proj(mc):
        for m4 in range(4):
            m = mc * 4 + m4
            ot = sb_out.tile([P, C], F32, tag="out", name=f"out{m}")

            def unit(m, m4, ot, cc, p):
                def run():
                    key = (m, cc)
                    if key not in proj_ps:
                        proj_ps[key] = ps_small.tile(
                            [P, 512], F32, tag="ps_small", name=f"pps{m}_{cc}"
                        )
                    nc.tensor.matmul(
                        proj_ps[key],
                        att_tiles[(p, mc)][:, m4 * P:(m4 + 1) * P],
                        wp_sb[p][:, cc * 512:(cc + 1) * 512],
                        start=(p == 0), stop=(p == PAIRS - 1),
                    )
                    if p == PAIRS - 1:
                        nc.vector.tensor_copy(
                            ot[:, cc * 512:(cc + 1) * 512], proj_ps.pop(key)
                        )
                return run

            def dma_unit(m, ot):
                def run():
                    nc.sync.dma_start(out=out_d[m * P:(m + 1) * P, :], in_=ot)
                return run

            for cc in range(2):
                for p in range(PAIRS):
                    work_q.append(unit(m, m4, ot, cc, p))
            work_q.append(dma_unit(m, ot))

    # --- one pair: continuous 64-tile loop in 2-tile blocks ---
    def emit_pair(p, qT, kT, budget, v_jit, after_att=None):
        ses = {}
        av = sm = None

        def emit_sc(t):
            mc, n = divmod(t, NT)
            sc = ps_sc.tile([P, 1024], F32, tag="sc", name=f"sc{p}_{t}")
            for h in range(2):
                lo, hi = h * DH, (h + 1) * DH
                nc.tensor.matmul(
                    sc[:, h * 512:(h + 1) * 512],
                    kT[lo:hi, n * P:(n + 1) * P],
                    qT[lo:hi, mc * 512:(mc + 1) * 512],
                    start=True, stop=True, skip_group_check=True,
                )
            return sc

        def emit_exp(t, sc):
            se = sb_se.tile([P, 1024], BF16, tag="se", name=f"se{p}_{t}")
            nc.scalar.activation(se, sc, EXP, scale=float(SCALE))
            ses[t] = se

        def emit_av(t):
            nonlocal av
            mc, n = divmod(t, NT)
            if n == 0:
                av = ps_av.tile([P, 512], F32, tag="av", name=f"av{p}_{mc}")
            se = ses[t]
            first, last = (n == 0), (n == NT - 1)
            for h in range(2):
                nc.tensor.matmul(
                    av[h * DH:(h + 1) * DH, :],
                    v_sb[n][:, p * P + h * DH:p * P + (h + 1) * DH],
                    se[:, h * 512:(h + 1) * 512],
                    start=first, stop=last, skip_group_check=True,
                )

        def emit_sm(t):
            nonlocal sm
            mc, n = divmod(t, NT)
            if n == 0:
                sm = ps_sm.tile([P, 512], F32, tag="sm", name=f"sm{p}_{mc}")
            se = ses.pop(t)
            first, last = (n == 0), (n == NT - 1)
            for h in range(2):
                nc.tensor.matmul(
                    sm[h * DH:(h + 1) * DH, :],
                    ones_bf,
                    se[:, h * 512:(h + 1) * 512],
                    start=first, stop=last, skip_group_check=True,
                )
            if last:
                rc = sb_rc.tile([P, 512], F32, tag="rc", name=f"rc{p}_{mc}")
                nc.vector.reciprocal_approx_fast(rc, sm)
                att = sb_att.tile([P, 512], BF16, tag="att", name=f"att{p}_{mc}")
                nc.vector.tensor_tensor(att, av, rc, op=mybir.AluOpType.mult)
                att_tiles[(p, mc)] = att
                if after_att is not None:
                    after_att(mc)

        n_blocks = 4 * NT // 2 + 1   # 33 blocks: 32 produce + lag tail
        for b in range(n_blocks):
            t0, t1 = 2 * b, 2 * b + 1
            if t0 < 4 * NT:
                scs = (emit_sc(t0), emit_sc(t1))
                emit_exp(t0, scs[0])
                emit_exp(t1, scs[1])
                if v_jit and t0 < NT:
                    emit_v(t0)
                    emit_v(t1)
            if b >= 1:
                s0, s1 = t0 - LAG, t1 - LAG
                emit_av(s0)
                emit_av(s1)
                emit_sm(s0)
                emit_sm(s1)
            if v_jit and t0 < NT:
                drain(1)        # keep deferred qT chunks flowing in mc0
            else:
                drain(2 * budget)

    # --- main schedule ---
    qkT_cur = emit_qkT_lead(0)
    qkT_next = push_qkT(1)
    emit_pair(0, *qkT_cur, budget=2, v_jit=True)

    qkT_cur, qkT_next = qkT_next, push_qkT(2)
    emit_pair(1, *qkT_cur, budget=2, v_jit=False)

    qkT_cur, qkT_next = qkT_next, push_qkT(3)
    emit_pair(2, *qkT_cur, budget=2, v_jit=False)

    qkT_cur = qkT_next
    emit_pair(3, *qkT_cur, budget=4, v_jit=False,
              after_att=lambda mc: push_proj(mc))
    drain(len(work_q))


def build_nc():
    from contextlib import ExitStack

    nc = bacc.Bacc("TRN2", target_bir_lowering=False, debug=False, num_devices=8)
    with tile.TileContext(nc) as tc:
        with ExitStack() as ctx:
            _emit(nc, tc, ctx)
    nc.compile()
    return nc


_NC = None


def _in_maps(x, W_qkv, W_proj):
    bf = ml_dtypes.bfloat16
    in_maps = []
    for c in range(8):
        b, h0 = c // 2, (c % 2) * HPC * DH  # h0 = col offset (0 or 512)
        xt = np.ascontiguousarray(x[b].T)                       # [C, N]
        xtc = (xt.reshape(KT, P, MC, 512).transpose(2, 1, 0, 3)
               .reshape(MC * P, KT * 512))
        wqk_cat = np.concatenate(
            [W_qkv[:, h0:h0 + 512], W_qkv[:, C + h0:C + h0 + 512]], axis=1
        )                                                       # [C, 1024]
        wqk3 = (wqk_cat.reshape(KT, P, 8, P).transpose(1, 2, 0, 3)
                .reshape(P, 8 * 1024))
        wv = W_qkv[:, 2 * C + h0:2 * C + h0 + 512]              # [C, 512]
        wv2 = wv.reshape(KT, P, 512).transpose(1, 0, 2).reshape(P, KT * 512)
        in_maps.append({
            "xTc": np.ascontiguousarray(xtc).astype(bf),
            "wqk": np.ascontiguousarray(wqk3).astype(bf),
            "wv": np.ascontiguousarray(wv2).astype(bf),
            "wp": np.ascontiguousarray(W_proj[h0:h0 + 512, :]).astype(bf),
        })
    return in_maps


def kernel(x, W_qkv, b_qkv, W_proj, b_proj):
    global _NC
    assert np.all(b_qkv == 0.0), "kernel assumes zero qkv bias"
    x = np.asarray(x, np.float32)
    W_qkv = np.asarray(W_qkv, np.float32)
    W_proj = np.asarray(W_proj, np.float32)
    b_proj = np.asarray(b_proj, np.float32)
    if _NC is None:
        _NC = build_nc()
    res = run_bass_kernel_spmd(_NC, _in_maps(x, W_qkv, W_proj), list(range(8)))
    out = np.empty((4, N, C), np.float32)
    for b in range(4):
        out[b] = res.results[2 * b]["out"] + res.results[2 * b + 1]["out"] + b_proj
    return out


# revision 26
# speedup vs baseline: 1.0815x; 1.0248x over previous
"""Multi-head attention block (B=4, N=2048, C=1024, H=16) on 8 trn2 cores.

Sharding: core c handles batch c//2 and heads (c%2)*8 .. (c%2)*8+8
(data parallel on B, tensor parallel on heads). Each core computes
qkv projections for its 8 heads, attention, and a partial output
projection (row-parallel over W_proj); the host sums the two partial
projections per batch and adds b_proj. The host also pre-transposes /
re-tiles x and the weights into DMA-friendly layouts (2-8KB per-partition
contiguous rows) and pre-casts to bf16 — pure data layout/sharding prep.

Per-core dataflow (layouts chosen so no on-device transposes are
needed):
  qT/kT[hd, m] = Wqk.T @ x.T   (W-stationary, bf16, psum-accum over k)
  v[n, hd]     = x @ Wv        (xT-stationary, bf16)
  St[n, m]     = k @ q.T       (kT-stationary, bf16, 2-head row-tiled
                                concurrent pair on the PE array)
  E = exp(St/8)                (ScalarE, fused scale, 1024-wide PSUM
                                reads across both heads' banks, bf16 out)
  av[d, m]     = v.T @ E       (bf16, 2-head col-tiled concurrent pair,
                                psum-accum over n)
  sums[m]      = ones64.T @ E  (replicated across 64 partitions by the
                                PE so no partition-broadcast is needed)
  att[d, m]    = av * approx_recip(sums)   (DVE)
  out_part     = att.T @ Wp    (bf16, psum-accum over head pairs)

Scheduling (v4): each pair runs one continuous 64-tile loop in 2-tile
blocks ordered [SC,SC | exp,exp | AV,AV,AV,AV,SM,SM,SM,SM | qp burst]
to minimize PE array-tiling mode switches (row->col->full cycles cost
~100ns each); qkv-for-next-pair and proj matmuls are spread via a work
queue; input DMAs are priority-ordered, big-packet, and partition-split
for queue parallelism; the exp table is preloaded at t=0.
"""

from collections import deque

import numpy as np
import ml_dtypes

import concourse.bass as bass
import concourse.mybir as mybir
import concourse.tile as tile
from concourse import bacc
from concourse.bass_utils import run_bass_kernel_spmd

F32 = mybir.dt.float32
BF16 = mybir.dt.bfloat16
EXP = mybir.ActivationFunctionType.Exp

N = 2048          # sequence length
C = 1024          # model dim
DH = 64           # head dim
HPC = 8           # heads per core
P = 128           # partitions
NT = N // P       # 16 n/m tiles
KT = C // P       # 8 contraction tiles for qkv
MC = N // 512     # 4 m-chunks of 512
PAIRS = HPC // 2  # 4 head pairs
SCALE = 1.0 / np.sqrt(DH)
LAG = 2           # tiles the av/sm consumer trails the sc/exp producer


def _emit(nc, tc, ctx):
    # host-retiled inputs (see _in_maps):
    #  xTc: [4*128, 4096]  row mc*128+p, col k*512+j  = x[mc*512+j, k*128+p]
    #  wqk: [128, 8192]    row p, col ct*1024+k*128+c = Wqk_cat[k*128+p, ct*128+c]
    #  wv:  [128, 4096]    row p, col k*512+c         = Wv[k*128+p, c]
    #  wp:  [512, 1024]    as-is
    xTc_d = nc.dram_tensor("xTc", [MC * P, KT * 512], BF16, kind="ExternalInput").ap()
    wqk_d = nc.dram_tensor("wqk", [P, 8 * 1024], BF16, kind="ExternalInput").ap()
    wv_d = nc.dram_tensor("wv", [P, KT * 512], BF16, kind="ExternalInput").ap()
    wp_d = nc.dram_tensor("wp", [HPC * DH, C], BF16, kind="ExternalInput").ap()
    out_d = nc.dram_tensor("out", [N, C], BF16, kind="ExternalOutput").ap()

    # --- pools ---
    consts = ctx.enter_context(tc.tile_pool(name="consts", bufs=1))
    sb_xT = ctx.enter_context(tc.tile_pool(name="sb_xT", bufs=MC))
    sb_wqk = ctx.enter_context(tc.tile_pool(name="sb_wqk", bufs=5))
    sb_wv = ctx.enter_context(tc.tile_pool(name="sb_wv", bufs=1))
    sb_wp = ctx.enter_context(tc.tile_pool(name="sb_wp", bufs=PAIRS))
    sb_v = ctx.enter_context(tc.tile_pool(name="sb_v", bufs=NT))
    sb_qkT = ctx.enter_context(tc.tile_pool(name="sb_qkT", bufs=4))
    sb_se = ctx.enter_context(tc.tile_pool(name="sb_se", bufs=6))
    sb_rc = ctx.enter_context(tc.tile_pool(name="sb_rc", bufs=2))
    sb_att = ctx.enter_context(tc.tile_pool(name="sb_att", bufs=PAIRS * MC))
    sb_out = ctx.enter_context(tc.tile_pool(name="sb_out", bufs=NT))

    ps_sc = ctx.enter_context(tc.tile_pool(name="ps_sc", bufs=2, space="PSUM"))
    ps_av = ctx.enter_context(tc.tile_pool(name="ps_av", bufs=1, space="PSUM"))
    ps_sm = ctx.enter_context(tc.tile_pool(name="ps_sm", bufs=1, space="PSUM"))
    ps_small = ctx.enter_context(tc.tile_pool(name="ps_small", bufs=2, space="PSUM"))

    # --- constants + exp activation-table preload (hides the ~2.7us
    # ACT_TABLE_LOAD under the initial DMA wait) ---
    ones_bf = consts.tile([P, DH], BF16)
    nc.vector.memset(ones_bf, 1.0)
    warm_in = consts.tile([P, 8], F32)
    warm_out = consts.tile([P, 8], BF16)
    nc.vector.memset(warm_in, 0.0)
    nc.scalar.activation(warm_out, warm_in, EXP, scale=1.0)

    # --- input DMAs, priority-ordered; the critical first tiles are
    # partition-split so several DMA queues move them in parallel ---
    xTc = [sb_xT.tile([P, KT * 512], BF16, tag="xT", name=f"xTc{mc}")
           for mc in range(MC)]
    wqk_loaded = {}

    def load_wqk(ct, split=1):
        w = sb_wqk.tile([P, 8 * P], BF16, tag="wqk", name=f"wqk{ct}")
        step = P // split
        for i in range(split):
            sl = slice(i * step, (i + 1) * step)
            nc.sync.dma_start(out=w[sl, :], in_=wqk_d[sl, ct * 1024:(ct + 1) * 1024])
        wqk_loaded[ct] = w

    for i in range(4):
        sl = slice(i * 32, (i + 1) * 32)
        nc.sync.dma_start(out=xTc[0][sl, :], in_=xTc_d[sl, :])
    load_wqk(PAIRS, split=2)   # k weights, pair 0 (kT chains run first)
    load_wqk(0, split=2)       # q weights, pair 0
    wv_all = sb_wv.tile([P, KT * 512], BF16, tag="wv", name="wv")
    nc.sync.dma_start(out=wv_all, in_=wv_d)
    for mc in range(1, MC):
        nc.sync.dma_start(out=xTc[mc], in_=xTc_d[mc * P:(mc + 1) * P, :])
    wp_sb = []
    for p in range(PAIRS):
        wb = sb_wp.tile([P, C], BF16, tag="wp", name=f"wp{p}")
        nc.sync.dma_start(out=wb, in_=wp_d[p * P:(p + 1) * P, :])
        wp_sb.append(wb)

    def xT(k, col0, w):
        """AP over x.T[k*128:(k+1)*128, col0:col0+w] in the mc-chunked tile."""
        mc, j = divmod(col0, 512)
        assert j + w <= 512
        return xTc[mc][:, k * 512 + j:k * 512 + j + w]

    # --- work queue of small PE bursts, drained per block ---
    work_q = deque()

    def drain(k):
        for _ in range(k):
            if work_q:
                work_q.popleft()()

    # --- v production (JIT during p0 mc0) ---
    v_sb = [None] * NT

    def emit_v(m):
        ps = ps_small.tile([P, 512], F32, tag="ps_small", name=f"vps{m}")
        for k in range(KT):
            nc.tensor.matmul(
                ps, xT(k, m * P, P), wv_all[:, k * 512:(k + 1) * 512],
                start=(k == 0), stop=(k == KT - 1),
            )
        vt = sb_v.tile([P, HPC * DH], BF16, tag="v", name=f"v{m}")
        nc.vector.tensor_copy(vt, ps)
        v_sb[m] = vt

    # --- qkT production: pair 0 up front; pairs 1-3 spread via work_q ---
    def wts(ct, k):
        return wqk_loaded[ct][:, k * P:(k + 1) * P]

    def emit_qkT_chain(ct, dst, mc):
        ps = ps_small.tile([P, 512], F32, tag="ps_small", name=f"qkps{ct}_{mc}")
        for k in range(KT):
            nc.tensor.matmul(
                ps, wts(ct, k), xT(k, mc * 512, 512),
                start=(k == 0), stop=(k == KT - 1),
            )
        nc.vector.tensor_copy(dst[:, mc * 512:(mc + 1) * 512], ps)

    def emit_qkT_lead(p):
        """kT fully + qT chunks 0-1 inline (the minimum to start attention
        and run through tile 31); qT chunks 2-3 go to the work queue."""
        qT = sb_qkT.tile([P, N], BF16, tag="qkT", name=f"qT{p}")
        kT = sb_qkT.tile([P, N], BF16, tag="qkT", name=f"kT{p}")
        for mc in range(MC):
            emit_qkT_chain(PAIRS + p, kT, mc)
        emit_qkT_chain(p, qT, 0)
        emit_qkT_chain(p, qT, 1)
        chain_ps = {}

        def unit(mc, k):
            def run():
                if mc not in chain_ps:
                    chain_ps[mc] = ps_small.tile(
                        [P, 512], F32, tag="ps_small", name=f"qkps{p}_{mc}"
                    )
                nc.tensor.matmul(
                    chain_ps[mc], wts(p, k), xT(k, mc * 512, 512),
                    start=(k == 0), stop=(k == KT - 1),
                )
                if k == KT - 1:
                    nc.vector.tensor_copy(
                        qT[:, mc * 512:(mc + 1) * 512], chain_ps.pop(mc)
                    )
            return run

        for mc in range(2, MC):
            for k in range(KT):
                work_q.append(unit(mc, k))
        return qT, kT

    def push_qkT(p):
        load_wqk(PAIRS + p)
        load_wqk(p)
        qT = sb_qkT.tile([P, N], BF16, tag="qkT", name=f"qT{p}")
        kT = sb_qkT.tile([P, N], BF16, tag="qkT", name=f"kT{p}")
        chain_ps = {}

        def unit(ct, dst, mc, k):
            def run():
                key = (ct, mc)
                if key not in chain_ps:
                    chain_ps[key] = ps_small.tile(
                        [P, 512], F32, tag="ps_small", name=f"qkps{ct}_{mc}"
                    )
                nc.tensor.matmul(
                    chain_ps[key], wts(ct, k), xT(k, mc * 512, 512),
                    start=(k == 0), stop=(k == KT - 1),
                )
                if k == KT - 1:
                    nc.vector.tensor_copy(
                        dst[:, mc * 512:(mc + 1) * 512], chain_ps.pop(key)
                    )
            return run

        for ct, dst in [(PAIRS + p, kT), (p, qT)]:
            for mc in range(MC):
                for k in range(KT):
                    work_q.append(unit(ct, dst, mc, k))
        return qT, kT

    # --- proj, spread via work_q in two pair-halves: {0,1} contribution
    # queued once pair 1's att is ready (drains through pair 2), {2,3}
    # queued at pair 3 (drains inline; only mc3's half-chains trail) ---
    att_tiles = {}
    proj_ps = {}
    ot_tiles = {}

    def push_proj_half(mc, plo):
        for m4 in range(4):
            m = mc * 4 + m4
            if plo == 0:
                ot_tiles[m] = sb_out.tile([P, C], BF16, tag="out", name=f"out{m}")
            ot = ot_tiles[m]

            def unit(m, m4, ot, cc, p):
                def run():
                    key = (m, cc)
                    if key not in proj_ps:
                        proj_ps[key] = ps_small.tile(
                            [P, 512], F32, tag="ps_small", name=f"pps{m}_{cc}_{p}"
                        )
                    nc.tensor.matmul(
                        proj_ps[key],
                        att_tiles[(p, mc)][:, m4 * P:(m4 + 1) * P],
                        wp_sb[p][:, cc * 512:(cc + 1) * 512],
                        start=(p == plo), stop=(p == plo + 1),
                    )
                    if p == plo + 1:
                        ps = proj_ps.pop(key)
                        sl = ot[:, cc * 512:(cc + 1) * 512]
                        if plo == 0:
                            nc.vector.tensor_copy(sl, ps)
                        else:
                            nc.vector.tensor_tensor(
                                sl, ps, sl, op=mybir.AluOpType.add
                            )
                return run

            def dma_unit(m, ot):
                def run():
                    nc.sync.dma_start(out=out_d[m * P:(m + 1) * P, :], in_=ot)
                return run

            for cc in range(2):
                for p in (plo, plo + 1):
                    work_q.append(unit(m, m4, ot, cc, p))
            if plo == 2:
                work_q.append(dma_unit(m, ot))

    # --- one pair: continuous 64-tile loop in 2-tile blocks ---
    def emit_pair(p, qT, kT, budget, v_jit, after_att=None):
        ses = {}
        av = sm = None

        def emit_sc(t):
            mc, n = divmod(t, NT)
            sc = ps_sc.tile([P, 1024], F32, tag="sc", name=f"sc{p}_{t}")
            for h in range(2):
                lo, hi = h * DH, (h + 1) * DH
                nc.tensor.matmul(
                    sc[:, h * 512:(h + 1) * 512],
                    kT[lo:hi, n * P:(n + 1) * P],
                    qT[lo:hi, mc * 512:(mc + 1) * 512],
                    start=True, stop=True, skip_group_check=True,
                )
            return sc

        def emit_exp(t, sc):
            se = sb_se.tile([P, 1024], BF16, tag="se", name=f"se{p}_{t}")
            nc.scalar.activation(se, sc, EXP, scale=float(SCALE))
            ses[t] = se

        def emit_av(t):
            nonlocal av
            mc, n = divmod(t, NT)
            if n == 0:
                av = ps_av.tile([P, 512], F32, tag="av", name=f"av{p}_{mc}")
            se = ses[t]
            first, last = (n == 0), (n == NT - 1)
            for h in range(2):
                nc.tensor.matmul(
                    av[h * DH:(h + 1) * DH, :],
                    v_sb[n][:, p * P + h * DH:p * P + (h + 1) * DH],
                    se[:, h * 512:(h + 1) * 512],
                    start=first, stop=last, skip_group_check=True,
                )

        def emit_sm(t):
            nonlocal sm
            mc, n = divmod(t, NT)
            if n == 0:
                sm = ps_sm.tile([P, 512], F32, tag="sm", name=f"sm{p}_{mc}")
            se = ses.pop(t)
            first, last = (n == 0), (n == NT - 1)
            for h in range(2):
                nc.tensor.matmul(
                    sm[h * DH:(h + 1) * DH, :],
                    ones_bf,
                    se[:, h * 512:(h + 1) * 512],
                    start=first, stop=last, skip_group_check=True,
                )
            if last:
                rc = sb_rc.tile([P, 512], F32, tag="rc", name=f"rc{p}_{mc}")
                nc.vector.reciprocal_approx_fast(rc, sm)
                att = sb_att.tile([P, 512], BF16, tag="att", name=f"att{p}_{mc}")
                nc.vector.tensor_tensor(att, av, rc, op=mybir.AluOpType.mult)
                att_tiles[(p, mc)] = att
                if after_att is not None:
                    after_att(mc)

        n_blocks = 4 * NT // 2 + 1   # 33 blocks: 32 produce + lag tail
        for b in range(n_blocks):
            t0, t1 = 2 * b, 2 * b + 1
            if t0 < 4 * NT:
                if v_jit and t0 < NT:
                    emit_v(t0)
                    emit_v(t1)
                scs = (emit_sc(t0), emit_sc(t1))
                emit_exp(t0, scs[0])
                emit_exp(t1, scs[1])
            if b >= 1:
                s0, s1 = t0 - LAG, t1 - LAG
                emit_av(s0)
                emit_av(s1)
                emit_sm(s0)
                emit_sm(s1)
            if not (v_jit and t0 < NT):
                drain(2 * budget)

    # --- main schedule ---
    qkT_cur = emit_qkT_lead(0)
    qkT_next = push_qkT(1)
    emit_pair(0, *qkT_cur, budget=2, v_jit=True)

    qkT_cur, qkT_next = qkT_next, push_qkT(2)
    emit_pair(1, *qkT_cur, budget=2, v_jit=False,
              after_att=lambda mc: push_proj_half(mc, 0))

    qkT_cur, qkT_next = qkT_next, push_qkT(3)
    emit_pair(2, *qkT_cur, budget=2, v_jit=False)

    qkT_cur = qkT_next
    emit_pair(3, *qkT_cur, budget=2, v_jit=False,
              after_att=lambda mc: push_proj_half(mc, 2))
    drain(len(work_q))


def build_nc():
    from contextlib import ExitStack

    nc = bacc.Bacc("TRN2", target_bir_lowering=False, debug=False, num_devices=8)
    with tile.TileContext(nc) as tc:
        with ExitStack() as ctx:
            _emit(nc, tc, ctx)
    nc.compile()
    return nc


_NC = None


def _in_maps(x, W_qkv, W_proj):
    bf = ml_dtypes.bfloat16
    in_maps = []
    for c in range(8):
        b, h0 = c // 2, (c % 2) * HPC * DH  # h0 = col offset (0 or 512)
        xt = np.ascontiguousarray(x[b].T)                       # [C, N]
        xtc = (xt.reshape(KT, P, MC, 512).transpose(2, 1, 0, 3)
               .reshape(MC * P, KT * 512))
        wqk_cat = np.concatenate(
            [W_qkv[:, h0:h0 + 512], W_qkv[:, C + h0:C + h0 + 512]], axis=1
        )                                                       # [C, 1024]
        wqk3 = (wqk_cat.reshape(KT, P, 8, P).transpose(1, 2, 0, 3)
                .reshape(P, 8 * 1024))
        wv = W_qkv[:, 2 * C + h0:2 * C + h0 + 512]              # [C, 512]
        wv2 = wv.reshape(KT, P, 512).transpose(1, 0, 2).reshape(P, KT * 512)
        in_maps.append({
            "xTc": np.ascontiguousarray(xtc).astype(bf),
            "wqk": np.ascontiguousarray(wqk3).astype(bf),
            "wv": np.ascontiguousarray(wv2).astype(bf),
            "wp": np.ascontiguousarray(W_proj[h0:h0 + 512, :]).astype(bf),
        })
    return in_maps


def kernel(x, W_qkv, b_qkv, W_proj, b_proj):
    global _NC
    assert np.all(b_qkv == 0.0), "kernel assumes zero qkv bias"
    x = np.asarray(x, np.float32)
    W_qkv = np.asarray(W_qkv, np.float32)
    W_proj = np.asarray(W_proj, np.float32)
    b_proj = np.asarray(b_proj, np.float32)
    if _NC is None:
        _NC = build_nc()
    res = run_bass_kernel_spmd(_NC, _in_maps(x, W_qkv, W_proj), list(range(8)))
    out = np.empty((4, N, C), np.float32)
    for b in range(4):
        out[b] = (res.results[2 * b]["out"].astype(np.float32)
                  + res.results[2 * b + 1]["out"].astype(np.float32) + b_proj)
    return out
